# revision 9
# baseline (speedup 1.0000x reference)
"""Multi-head attention block (B=4, S=2048, D=1024, H=16) on 8 TRN2 NeuronCores.

Sharding: core c handles batch b = c//2 and head-group hg = c%2 (8 heads,
a 512-wide slice of the qkv projections). No collectives: each core
computes a [D, S] transposed partial of the output projection for its
head group; the host sums the two head-group partials per batch, adds
the output bias (with bv @ Wo folded in -- softmax rows sum to 1, so the
V bias passes through attention as a constant), and transposes back.

Per-core dataflow (bf16 compute, f32 PSUM accumulation). The ENTIRE
kernel runs in the PE's 64x128 row-tiled mode (tile T0 = SBUF partitions
0-63, tile T8 = 64-127) so there are no tiling-mode drains (~130ns each)
anywhere:
  - scores^T [k, q]: head A of each pair contracts its 64 hd dims in T0
    while head B runs CONCURRENTLY in T8 -- 2x the K=128 zero-padded
    scheme (the second matmul of a pair issues ~4ns after the first).
  - projections / attn@V / out-proj have full 128-deep contractions;
    each 128-tile is split into lo/hi 64-halves that run as a concurrent
    T0/T8 pair into two PSUM banks (same wall time as one K=128 matmul),
    merged during the DVE PSUM->SBUF evacuation (tensor_tensor add, or
    scalar_tensor_tensor when a projection bias rides along).
  - exp on ACT (PSUM -> SBUF bf16, scale=1/8); softmax denominators ride
    the attn@V matmuls as a 65th stationary column of ones;
    normalization via DVE reciprocal + GPSIMD partition-broadcast + DVE
    multiply.
  - out^T = Wo^T O^T -> [D, S] f32 -> DMA out.

PSUM budget (8 banks): "sc" ring 2 x [128,1024] (4 banks, shared by
scores pairs AND all projection/out-proj lo/hi pairs) + "o" ring 2 x
[65,1024] (4 banks, the per-chunk attn@V lo/hi accumulator pairs).
"""

import numpy as np
import ml_dtypes

import concourse.bass as bass
import concourse.bacc as bacc
import concourse.mybir as mybir
from concourse.tile import TileContext
from concourse.bass import ds

F32 = mybir.dt.float32
BF16 = mybir.dt.bfloat16
EXP = mybir.ActivationFunctionType.Exp
ADD = mybir.AluOpType.add

B, S, D, H, HD = 4, 2048, 1024, 16, 64
N_CORES = 8
HPC = H // (N_CORES // B)          # heads per core = 8
DV = HPC * HD                      # 512


def build_attn_core(S=2048, D=1024, HPC=8, HD=64):
    DV = HPC * HD            # head-group width
    NPAIR = HPC // 2         # head pairs; DV = NPAIR * 128
    NDT = D // 128           # din tiles
    NKT = S // 128           # key tiles
    QC = 512                 # q chunk
    NQC = S // QC
    SC = 512                 # s chunk for projections
    NSC = S // SC
    SCALE = HD ** -0.5

    nc = bacc.Bacc("TRN2", target_bir_lowering=False)
    q_ext = nc.dram_tensor("queryT", [D, S], BF16, kind="ExternalInput")
    k_ext = nc.dram_tensor("keyT", [D, S], BF16, kind="ExternalInput")
    v_ext = nc.dram_tensor("valueT", [D, S], BF16, kind="ExternalInput")
    wq_ext = nc.dram_tensor("Wq", [D, DV], BF16, kind="ExternalInput")
    wk_ext = nc.dram_tensor("Wk", [D, DV], BF16, kind="ExternalInput")
    wv_ext = nc.dram_tensor("Wv", [D, DV], BF16, kind="ExternalInput")
    wo_ext = nc.dram_tensor("Wo", [DV, D], BF16, kind="ExternalInput")
    bq_ext = nc.dram_tensor("bq", [DV], F32, kind="ExternalInput")
    bk_ext = nc.dram_tensor("bk", [DV], F32, kind="ExternalInput")
    out_ext = nc.dram_tensor("out", [D, S], F32, kind="ExternalOutput")

    with TileContext(nc) as tc:
        with (
            tc.tile_pool(name="const", bufs=1) as cpool,
            tc.tile_pool(name="big", bufs=1) as big,
            tc.tile_pool(name="pt", bufs=8) as ptpool,
            tc.tile_pool(name="vl", bufs=3) as vlpool,
            tc.tile_pool(name="rec", bufs=2) as recpool,
            tc.tile_pool(name="oun", bufs=4) as ounpool,
            tc.tile_pool(name="stage", bufs=2) as stage,
            tc.tile_pool(name="mrg", bufs=3) as mrg,
            tc.tile_pool(name="scps", bufs=2, space="PSUM") as scps,
            tc.tile_pool(name="ops", bufs=2, space="PSUM") as opool,
        ):
            # -------- biases / ones first (tiny DMAs; needed by the DVE
            # bias-folded PSUM evacuations of every Q/K projection chunk).
            bq_col = cpool.tile([128, NPAIR], F32, tag="bqc")
            bk_col = cpool.tile([128, NPAIR], F32, tag="bkc")
            ones_pad = cpool.tile([128, SC], BF16, tag="onesp")
            nc.vector.memset(ones_pad[:], 0.0)
            nc.vector.memset(ones_pad[0:1, :], 1.0)
            nc.sync.dma_start(bq_col[:], bq_ext.rearrange("(t p) -> p t", p=128))
            nc.sync.dma_start(bk_col[:], bk_ext.rearrange("(t p) -> p t", p=128))

            # -------- big inputs: emission order = DMA priority ----------
            xqT = big.tile([128, NDT, S], BF16, tag="xqT")
            xkT = big.tile([128, NDT, S], BF16, tag="xkT")
            wq_sb = big.tile([128, NDT, DV], BF16, tag="wq")
            wk_sb = big.tile([128, NDT, DV], BF16, tag="wk")
            wv_sb = big.tile([128, NDT, DV], BF16, tag="wv")
            wo_sb = big.tile([128, NPAIR, D], BF16, tag="wo")
            nc.sync.dma_start(wq_sb[:], wq_ext.rearrange("(t p) n -> p t n", p=128))
            nc.sync.dma_start(xqT[:], q_ext.rearrange("(t p) s -> p t s", p=128))
            nc.sync.dma_start(wk_sb[:], wk_ext.rearrange("(t p) n -> p t n", p=128))
            nc.sync.dma_start(xkT[:], k_ext.rearrange("(t p) s -> p t s", p=128))
            nc.sync.dma_start(wv_sb[:], wv_ext.rearrange("(t p) n -> p t n", p=128))
            nc.sync.dma_start(wo_sb[:], wo_ext.rearrange("(t p) n -> p t n", p=128))

            # -------- persistent SBUF tensors ----------------------------
            # qT/kT hold both heads of each pair stacked on partitions (A on
            # 0-63, B on 64-127); the row-tiled scores matmuls contract each
            # 64-partition half independently -- no zero-padding needed.
            qT = big.tile([128, NPAIR, S], BF16, tag="qT")
            kT = big.tile([128, NPAIR, S], BF16, tag="kT")
            v_aug = big.tile([128, NKT, HPC * 65], BF16, tag="vaug")
            oT = big.tile([128, NPAIR, S], BF16, tag="xqT")  # reuse xqT memory (dead after Q-projs)

            # -------- work-chunk emitters --------------------------------
            # Every 128-deep contraction runs as a concurrent T0/T8 pair of
            # 64-deep matmuls into the lo/hi banks of one [128,1024] tile.
            def mm_pair(ps, lhsT, rhs, start, stop):
                nc.tensor.matmul(
                    ps[:, 0:512], lhsT[0:64], rhs[0:64], start=start, stop=stop
                )
                nc.tensor.matmul(
                    ps[:, 512:1024], lhsT[64:128], rhs[64:128], start=start, stop=stop
                )

            def q_proj_sc(t, sc, pool=None):
                pool, tag = pool or (scps, "sc")
                ps = pool.tile([128, 1024], F32, tag=tag)
                for dk in range(NDT):
                    mm_pair(
                        ps,
                        wq_sb[:, dk, ds(t * 128, 128)],
                        xqT[:, dk, ds(sc * SC, SC)],
                        start=(dk == 0),
                        stop=(dk == NDT - 1),
                    )
                tmp = mrg.tile([128, SC], F32, tag="mrg")
                nc.vector.tensor_copy(tmp[:], ps[:, 512:1024])
                nc.vector.scalar_tensor_tensor(
                    qT[:, t, ds(sc * SC, SC)],
                    ps[:, 0:512], bq_col[:, t : t + 1], tmp[:],
                    ADD, ADD,
                )

            def k_proj_sc(t, sc, pool=None):
                pool, tag = pool or (scps, "sc")
                ps = pool.tile([128, 1024], F32, tag=tag)
                for dk in range(NDT):
                    mm_pair(
                        ps,
                        wk_sb[:, dk, ds(t * 128, 128)],
                        xkT[:, dk, ds(sc * SC, SC)],
                        start=(dk == 0),
                        stop=(dk == NDT - 1),
                    )
                tmp = mrg.tile([128, SC], F32, tag="mrg")
                nc.vector.tensor_copy(tmp[:], ps[:, 512:1024])
                nc.vector.scalar_tensor_tensor(
                    kT[:, t, ds(sc * SC, SC)],
                    ps[:, 0:512], bk_col[:, t : t + 1], tmp[:],
                    ADD, ADD,
                )

            def v_proj_st(st):
                vl = vlpool.tile([128, NDT, 128], BF16, tag="vl")
                nc.sync.dma_start(
                    vl[:],
                    v_ext[:, ds(st * 128, 128)].rearrange("(t p) s -> p t s", p=128),
                )
                ps = scps.tile([128, 1024], F32, tag="sc")
                for dk in range(NDT):
                    mm_pair(
                        ps,
                        vl[:, dk, :],
                        wv_sb[:, dk, :],
                        start=(dk == 0),
                        stop=(dk == NDT - 1),
                    )
                tmp = mrg.tile([128, SC], F32, tag="mrg")
                nc.vector.tensor_copy(tmp[:], ps[:, 512:1024])
                dst = v_aug[:, st, :].rearrange("p (h c) -> p h c", c=65)
                nc.vector.tensor_tensor(
                    dst[:, :, 0:64],
                    ps[:, 0:512].rearrange("p (h c) -> p h c", c=64),
                    tmp[:].rearrange("p (h c) -> p h c", c=64),
                    ADD,
                )
                nc.vector.memset(dst[:, :, 64:65], 1.0)

            def outproj_dt(sc, dt2):
                po = scps.tile([128, 1024], F32, tag="sc")
                for ht in range(NPAIR):
                    mm_pair(
                        po,
                        wo_sb[:, ht, ds(dt2 * 128, 128)],
                        oT[:, ht, ds(sc * SC, SC)],
                        start=(ht == 0),
                        stop=(ht == NPAIR - 1),
                    )
                tmp = mrg.tile([128, SC], F32, tag="mrg")
                nc.vector.tensor_copy(tmp[:], po[:, 512:1024])
                ost = stage.tile([128, SC], F32, tag="ostage")
                nc.vector.tensor_tensor(ost[:], po[:, 0:512], tmp[:], ADD)
                nc.sync.dma_start(
                    out_ext[ds(dt2 * 128, 128), ds(sc * SC, SC)], ost[:]
                )

            # -------- HAM warm-up ----------------------------------------
            # ~7us of dependency-free matmuls on the ones tile: keeps the PE
            # busy during the initial DMA-only window so the HAM activity
            # monitor un-throttles the clock before the real projections.
            # Runs in the same 64x128 tiling mode as everything else.
            warm_rot = [(scps, "sc"), (opool, "o")]
            for wi in range(16):
                wpool, wtag = warm_rot[wi % 2]
                wps = wpool.tile([128, 1024], F32, tag=wtag)
                nc.tensor.matmul(
                    wps[:, 0:512], ones_pad[0:64, 0:128], ones_pad[0:64, :],
                    start=True, stop=True,
                )
                nc.tensor.matmul(
                    wps[:, 512:1024], ones_pad[0:64, 0:128], ones_pad[0:64, :],
                    start=True, stop=True,
                )

            # -------- projections needed before attention(0) -------------
            # Rotate PSUM chunks through both rings so the PE pipeline stays
            # dense (slot-wait gaps would reset the HAM activity window).
            rot = [(scps, "sc"), (opool, "o")]
            ri = 0
            for t in range(NPAIR):
                for sc in range(NSC):
                    q_proj_sc(t, sc, pool=rot[ri % 2]); ri += 1
            for sc in range(NSC):
                k_proj_sc(0, sc, pool=rot[ri % 2]); ri += 1

            # -------- attention ------------------------------------------
            SCALEF = SCALE

            deferred = []

            def normalize_one(o_sb, t, qc, hh):
                # Move the denominator row to partition 0 with a tiny
                # SBUF->SBUF DMA (partition-free, idle engines), broadcast,
                # then take the reciprocal on all 64 lanes in parallel.
                dn = recpool.tile([64, QC], F32, tag="rec")
                nc.sync.dma_start(dn[0:1, :], o_sb[64:65, :])
                bc = recpool.tile([64, QC], F32, tag="rec")
                nc.gpsimd.partition_broadcast(bc[:], dn[0:1, :])
                nc.vector.reciprocal_approx_fast(out=bc[:], in_=bc[:])
                nc.vector.tensor_mul(
                    oT[ds(hh * 64, 64), t, ds(qc * QC, QC)],
                    o_sb[0:64, :],
                    bc[:],
                )

            def attention_pair(t):
                for qc in range(NQC):
                    # filler thunks interleaved after each scores/exp step;
                    # lag = how many k-tiles attn@V trails the exp stream
                    fillers = {}
                    lag = 4
                    if t == 0 and qc == 0:
                        for st in range(NKT):
                            fillers.setdefault(min(1 + st, NKT - 1), []).append(
                                lambda st=st: v_proj_st(st)
                            )
                    if t + 1 < NPAIR and qc == NQC - 1:
                        for i in range(NSC):
                            fillers.setdefault(2 + 4 * i, []).append(
                                lambda tt=t + 1, sc=i: k_proj_sc(tt, sc)
                            )
                    if t == NPAIR - 1 and qc > 0:
                        for i in range(NDT):
                            fillers.setdefault(5 + i, []).append(
                                lambda sc=qc - 1, dt2=i: outproj_dt(sc, dt2)
                            )

                    # attn@V accumulators: lo/hi key-half partials for each
                    # head in the two banks of one [65,1024] tile.
                    oA = opool.tile([65, 1024], F32, tag="o")
                    oB = opool.tile([65, 1024], F32, tag="o")
                    pts = {}

                    def scores_exp(kt):
                        # Concurrent row-tiled pair: head A in T0, head B in
                        # T8, each contracting its own 64 hd dims.
                        sct = scps.tile([128, 2 * QC], F32, tag="sc")
                        nc.tensor.matmul(
                            sct[:, 0:QC],
                            kT[0:64, t, ds(kt * 128, 128)],
                            qT[0:64, t, ds(qc * QC, QC)],
                            start=True, stop=True,
                        )
                        nc.tensor.matmul(
                            sct[:, QC : 2 * QC],
                            kT[64:128, t, ds(kt * 128, 128)],
                            qT[64:128, t, ds(qc * QC, QC)],
                            start=True, stop=True,
                        )
                        pt = ptpool.tile([128, 2 * QC], BF16, tag="pt")
                        nc.scalar.activation(pt[:], sct[:], EXP, bias=0.0, scale=SCALEF)
                        pts[kt] = pt

                    def attn_v(kt):
                        # Each head's 128-key contraction runs as a
                        # concurrent T0/T8 pair over the key halves,
                        # accumulating into the lo/hi banks of its tile.
                        pt = pts.pop(kt)
                        st_, sp = (kt == 0), (kt == NKT - 1)
                        va = v_aug[:, kt, ds((2 * t) * 65, 65)]
                        vb = v_aug[:, kt, ds((2 * t + 1) * 65, 65)]
                        nc.tensor.matmul(
                            oA[:, 0:QC], va[0:64], pt[0:64, 0:QC],
                            start=st_, stop=sp,
                        )
                        nc.tensor.matmul(
                            oA[:, QC : 2 * QC], va[64:128], pt[64:128, 0:QC],
                            start=st_, stop=sp,
                        )
                        nc.tensor.matmul(
                            oB[:, 0:QC], vb[0:64], pt[0:64, QC : 2 * QC],
                            start=st_, stop=sp,
                        )
                        nc.tensor.matmul(
                            oB[:, QC : 2 * QC], vb[64:128], pt[64:128, QC : 2 * QC],
                            start=st_, stop=sp,
                        )

                    scores_exp(0)
                    for f in fillers.get(0, ()):
                        f()
                    for kt in range(1, NKT):
                        scores_exp(kt)
                        for f in fillers.get(kt, ()):
                            f()
                        if kt in (1, 3) and deferred:
                            normalize_one(*deferred.pop(0))
                        if kt - lag >= 0:
                            attn_v(kt - lag)
                    for kt in range(max(0, NKT - lag), NKT):
                        attn_v(kt)

                    # merge each accumulator's lo/hi key-half partials
                    # PSUM->SBUF now (frees the banks for the next chunk);
                    # the recip/broadcast/mul chains are DEFERRED into the
                    # next chunk's loop so they soak up DVE idle time there.
                    for o_ps, hh in ((oA, 0), (oB, 1)):
                        tmp_o = mrg.tile([65, QC], F32, tag="omrg")
                        nc.vector.tensor_copy(tmp_o[:], o_ps[:, QC : 2 * QC])
                        o_sb = ounpool.tile([65, QC], F32, tag="oun")
                        nc.vector.tensor_tensor(
                            o_sb[:], o_ps[:, 0:QC], tmp_o[:], ADD
                        )
                        if t == NPAIR - 1 and qc == NQC - 1:
                            normalize_one(o_sb, t, qc, hh)
                        else:
                            deferred.append((o_sb, t, qc, hh))

            for t in range(NPAIR):
                attention_pair(t)
            while deferred:
                normalize_one(*deferred.pop(0))
            # trailing output-projection chunk for the last s-chunk
            for dt2 in range(NDT):
                outproj_dt(NQC - 1, dt2)

    nc.finalize()
    return nc


_NC_CACHE = {}


def _get_nc():
    if "nc" not in _NC_CACHE:
        _NC_CACHE["nc"] = build_attn_core(S=S, D=D, HPC=HPC, HD=HD)
    return _NC_CACHE["nc"]


def _make_in_maps(query, key, value, Wq, bq, Wk, bk, Wv, Wo):
    bf = ml_dtypes.bfloat16
    in_maps = []
    for c in range(N_CORES):
        b, hg = c // 2, c % 2
        sl = slice(hg * DV, (hg + 1) * DV)
        in_maps.append(dict(
            queryT=np.ascontiguousarray(query[b].T).astype(bf),
            keyT=np.ascontiguousarray(key[b].T).astype(bf),
            valueT=np.ascontiguousarray(value[b].T).astype(bf),
            Wq=np.ascontiguousarray(Wq[:, sl]).astype(bf),
            Wk=np.ascontiguousarray(Wk[:, sl]).astype(bf),
            Wv=np.ascontiguousarray(Wv[:, sl]).astype(bf),
            Wo=np.ascontiguousarray(Wo[sl, :]).astype(bf),
            bq=np.ascontiguousarray(bq[sl]).astype(np.float32),
            bk=np.ascontiguousarray(bk[sl]).astype(np.float32),
        ))
    return in_maps


def _assemble(results, bo_eff):
    out = np.empty((B, S, D), dtype=np.float32)
    for b in range(B):
        part = results[2 * b]["out"] + results[2 * b + 1]["out"]   # [D, S]
        out[b] = part.T + bo_eff
    return out


def run(inputs, trace=False):
    """Run on 8 cores; returns (output, BassKernelResults)."""
    from concourse.bass_utils import run_bass_kernel_spmd

    inputs = {k: np.asarray(v) for k, v in inputs.items()}
    nc = _get_nc()
    in_maps = _make_in_maps(
        inputs["query"], inputs["key"], inputs["value"],
        inputs["Wq"], inputs["bq"], inputs["Wk"], inputs["bk"],
        inputs["Wv"], inputs["Wo"],
    )
    res = run_bass_kernel_spmd(
        nc, in_maps, core_ids=list(range(N_CORES)), trace=trace
    )
    # softmax rows sum to 1, so the V bias contributes bv @ Wo as a
    # constant: fold it into the output bias on the host (exact, f32).
    bo_eff = (
        np.asarray(inputs["bo"], dtype=np.float32)
        + np.asarray(inputs["bv"], dtype=np.float32)
        @ np.asarray(inputs["Wo"], dtype=np.float32)
    )
    out = _assemble(res.results, bo_eff)
    return out, res


def kernel(**inputs) -> np.ndarray:
    out, _ = run(inputs, trace=False)
    return out
